# revision 25
# baseline (speedup 1.0000x reference)
"""Multi-head attention (B=2, N=2048, D=1024, H=16) on 8 trn2 NeuronCores.

Sharding: DP2 (batch) x TP4 (head quarters).  Core c handles batch c//4 and
heads [4*(c%4), 4*(c%4)+4).  Per core:
  - QKV projection for its 256 local dims (x^T streamed, weights stationary)
  - causal flash-style attention (no max subtraction: scores/32 are tiny so
    exp is safe; softmax denominator comes from a ones-column fused into the
    z-accumulation matmul)
  - AllToAll of normalized z^T (bf16) within the 4-core batch group: each
    core receives, for every head, exactly its 512-query slice
  - output projection for the core's 512-row slice of the sequence
Host: slices/casts inputs, concatenates the 8 disjoint output slices.
"""

import os
import sys

for _p in ("/opt/trn_rl_repo", "/root/.axon_site/_ro/trn_rl_repo"):
    if os.path.isdir(_p) and _p not in sys.path:
        sys.path.append(_p)

import numpy as np
import ml_dtypes

import concourse.bass as bass
import concourse.mybir as mybir
import concourse.tile as tile
from concourse import bacc

B, N, D, H, HD = 2, 2048, 1024, 16, 64
NCORES, TP = 8, 4
DLOC = D // TP            # 256 local dims (4 heads) per core
P = 128
KT_X = D // P             # 8 contraction tiles for projections
NT = N // P               # 16 n-tiles
QB = 512                  # query block (PSUM bank width in fp32)
NQB = N // QB             # 4
NSLICE = N // TP          # 512 output rows per core
SCALE = 1.0 / 32.0        # 1/sqrt(D)

F32 = mybir.dt.float32
BF16 = mybir.dt.bfloat16
BF = ml_dtypes.bfloat16
Alu = mybir.AluOpType
Act = mybir.ActivationFunctionType


def build_bass():
    nc = bacc.Bacc("TRN2", num_devices=NCORES)

    xT = nc.dram_tensor("xT", [D, N], BF16, kind="ExternalInput")
    wq = nc.dram_tensor("wq", [D, DLOC], BF16, kind="ExternalInput")
    wk = nc.dram_tensor("wk", [D, DLOC], BF16, kind="ExternalInput")
    wv = nc.dram_tensor("wv", [D, DLOC], BF16, kind="ExternalInput")
    wo = nc.dram_tensor("wo", [D, D], BF16, kind="ExternalInput")
    bq = nc.dram_tensor("bq", [DLOC], F32, kind="ExternalInput")
    bk = nc.dram_tensor("bk", [DLOC], F32, kind="ExternalInput")
    bv = nc.dram_tensor("bv", [DLOC], F32, kind="ExternalInput")
    bo = nc.dram_tensor("bo", [D], F32, kind="ExternalInput")
    qoff = nc.dram_tensor("qoff", [1, 1], mybir.dt.uint32, kind="ExternalInput")
    out = nc.dram_tensor("out", [NSLICE, D], F32, kind="ExternalOutput")

    with tile.TileContext(nc) as tc:
        with (
            tc.tile_pool(name="persist", bufs=1) as persist,
            tc.tile_pool(name="wtp", bufs=4) as wtp,
            tc.tile_pool(name="small", bufs=4) as small,
            tc.tile_pool(name="psA", bufs=2, space="PSUM") as psA,
            tc.tile_pool(name="psS", bufs=2, space="PSUM") as psS,
            tc.tile_pool(name="psZ", bufs=4, space="PSUM") as psZ,
            tc.tile_pool(name="dram", bufs=1, space="DRAM") as dram,
        ):
            # ---- constant / input loads (weights first so projection can
            # start as soon as the first xT k-tile lands) ----
            wq_sb = persist.tile([P, KT_X, DLOC], BF16)
            nc.sync.dma_start(wq_sb, wq[:].rearrange("(kt p) m -> p kt m", p=P))
            wk_sb = persist.tile([P, KT_X, DLOC], BF16)
            nc.sync.dma_start(wk_sb, wk[:].rearrange("(kt p) m -> p kt m", p=P))
            wv_sb = persist.tile([P, KT_X, DLOC], BF16)
            nc.sync.dma_start(wv_sb, wv[:].rearrange("(kt p) m -> p kt m", p=P))
            xT_sb = persist.tile([P, KT_X, N], BF16)
            for kt in range(KT_X):
                nc.sync.dma_start(
                    xT_sb[:, kt], xT[kt * P:(kt + 1) * P, :]
                )

            bq_sb = small.tile([P, 2], F32)
            nc.sync.dma_start(bq_sb, bq[:].rearrange("(t p) -> p t", p=P))
            bqs_sb = persist.tile([P, 2], F32)
            nc.vector.tensor_scalar_mul(bqs_sb, bq_sb, SCALE)
            bk_sb = persist.tile([P, 2], F32)
            nc.sync.dma_start(bk_sb, bk[:].rearrange("(t p) -> p t", p=P))

            bv_row = small.tile([1, DLOC], F32)
            nc.sync.dma_start(bv_row, bv[:].rearrange("(a d) -> a d", a=1))
            bv_bc = persist.tile([P, DLOC], F32)
            nc.gpsimd.partition_broadcast(bv_bc, bv_row)

            bo_row = small.tile([1, D], F32)
            nc.sync.dma_start(bo_row, bo[:].rearrange("(a d) -> a d", a=1))
            bo_bc = persist.tile([P, D], F32)
            nc.gpsimd.partition_broadcast(bo_bc, bo_row)

            # staircase causal mask: mask[kk, c] = 1 iff c >= kk + 384;
            # view [:, 384-o : 384-o+width] gives "keep iff qq >= kk + o"
            mask_sb = persist.tile([P, QB + 384], BF16)
            nc.gpsimd.memset(mask_sb, 1.0)
            nc.gpsimd.affine_select(
                out=mask_sb,
                in_=mask_sb,
                compare_op=Alu.is_ge,
                fill=0.0,
                base=-384,
                pattern=[[1, QB + 384]],
                channel_multiplier=-1,
            )

            # ---- QKV projection helpers ----
            qT_sb = persist.tile([P, 2, N], BF16)
            kT_sb = persist.tile([P, 2, N], BF16)
            v_sb = persist.tile([P, NT, 4 * 65], BF16)
            nc.vector.memset(v_sb, 1.0)  # preset ones columns for denominators
            ones_sb = persist.tile([1, 64], F32)
            nc.vector.memset(ones_sb, 1.0)

            def qk_proj(mt):
                for qc in range(NQB):
                    psq = psA.tile([P, QB], F32, tag="proj", name="psq")
                    for kt in range(KT_X):
                        nc.tensor.matmul(
                            psq,
                            lhsT=wq_sb[:, kt, mt * P:(mt + 1) * P],
                            rhs=xT_sb[:, kt, qc * QB:(qc + 1) * QB],
                            start=(kt == 0),
                            stop=(kt == KT_X - 1),
                        )
                    nc.vector.tensor_scalar(
                        qT_sb[:, mt, qc * QB:(qc + 1) * QB],
                        psq,
                        SCALE,
                        bqs_sb[:, mt:mt + 1],
                        Alu.mult,
                        Alu.add,
                    )
                    psk = psA.tile([P, QB], F32, tag="proj", name="psk")
                    for kt in range(KT_X):
                        nc.tensor.matmul(
                            psk,
                            lhsT=wk_sb[:, kt, mt * P:(mt + 1) * P],
                            rhs=xT_sb[:, kt, qc * QB:(qc + 1) * QB],
                            start=(kt == 0),
                            stop=(kt == KT_X - 1),
                        )
                    nc.vector.tensor_scalar_add(
                        kT_sb[:, mt, qc * QB:(qc + 1) * QB],
                        psk,
                        bk_sb[:, mt:mt + 1],
                    )

            def v_proj():
                for nt in range(NT):
                    psv_full = psA.tile([P, QB], F32, tag="proj", name="psv")
                    psv = psv_full[:, :DLOC]
                    for kt in range(KT_X):
                        nc.tensor.matmul(
                            psv,
                            lhsT=xT_sb[:, kt, nt * P:(nt + 1) * P],
                            rhs=wv_sb[:, kt, :],
                            start=(kt == 0),
                            stop=(kt == KT_X - 1),
                        )
                    nc.vector.tensor_tensor(
                        v_sb[:, nt].rearrange("p (h x) -> p h x", x=65)[:, :, 0:64],
                        psv.rearrange("p (h x) -> p h x", x=64),
                        bv_bc.rearrange("p (h x) -> p h x", x=64),
                        Alu.add,
                    )

            # per-core dynamic sequence offset for the output projection
            qoff_sb = small.tile([1, 1], mybir.dt.uint32)
            nc.sync.dma_start(qoff_sb, qoff[:])
            qregs = nc.alloc_registers()
            nc.regs_load(qregs, qoff_sb[0:1, 0:1])
            qoff_sv = nc.snap(qregs, donate=True)

            # one AllGather per head pair so the first overlaps the second
            # pair's attention.  zallX rows: rank r block = global dims
            # [r*256 + pair*128, +128).
            zin = [dram.tile([P, N], BF16, name=f"zin{pr}") for pr in range(2)]
            zall = [dram.tile([TP * P, N], BF16, name=f"zall{pr}")
                    for pr in range(2)]

            # ---- attention for one head pair ----
            def attention(pr):
                for qb in range(NQB):
                    kt_max = (qb + 1) * 4
                    zps = [psZ.tile([65, QB], F32, tag="z", name=f"zp{hi}")
                           for hi in range(2)]
                    for kt in range(kt_max):
                        diag = kt >= qb * 4
                        o = kt * P - qb * QB if diag else 0
                        # score matmuls for the two heads back-to-back: they
                        # use disjoint PE row groups (contraction partitions
                        # 0-63 / 64-127) and run concurrently
                        sps, wts = [], []
                        for hi in range(2):
                            spf = psS.tile([P, QB], F32, tag="score", name="spf")
                            sp = spf[:, o:QB]
                            nc.tensor.matmul(
                                sp,
                                lhsT=kT_sb[hi * 64:(hi + 1) * 64, pr,
                                           kt * P:(kt + 1) * P],
                                rhs=qT_sb[hi * 64:(hi + 1) * 64, pr,
                                          qb * QB + o:(qb + 1) * QB],
                                start=True,
                                stop=True,
                                tile_position=(hi * 64, 0),
                            )
                            sps.append(sp)
                        for hi in range(2):
                            wt = wtp.tile([P, QB], BF16, tag="wt", name="wt")
                            nc.scalar.activation(wt[:, o:QB], sps[hi], Act.Exp)
                            if diag:
                                # only the o..o+128 strip straddles the diagonal
                                nc.vector.tensor_tensor(
                                    wt[:, o:o + P],
                                    wt[:, o:o + P],
                                    mask_sb[:, 384:384 + P],
                                    Alu.mult,
                                )
                            wts.append(wt)
                        for hi in range(2):
                            nc.tensor.matmul(
                                zps[hi][:, o:QB],
                                lhsT=v_sb[:, kt,
                                          (2 * pr + hi) * 65:(2 * pr + hi + 1) * 65],
                                rhs=wts[hi][:, o:QB],
                                start=(kt == 0),
                                stop=(kt == kt_max - 1),
                                skip_group_check=True,
                            )
                    for hi in range(2):
                        # denominator -> SBUF (approx_fast misbehaves on a
                        # PSUM source), reciprocal, gpsimd partition
                        # broadcast, normalize.  No PE in this chain, and
                        # the zin write rides the gpsimd queue so the sync
                        # queue's post-collective DMAs can't block it.
                        den = small.tile([1, QB], F32, tag="den", name="den")
                        nc.vector.tensor_copy(den, zps[hi][64:65, :])
                        recip = small.tile([1, QB], F32, tag="recip", name="recip")
                        nc.vector.reciprocal_approx_fast(recip, den)
                        rb = small.tile([64, QB], F32, tag="rb", name="rb")
                        nc.gpsimd.partition_broadcast(rb, recip)
                        zn = small.tile([64, QB], BF16, tag="zn", name="zn")
                        nc.vector.tensor_tensor(zn, zps[hi][0:64, :], rb, Alu.mult)
                        nc.gpsimd.dma_start(
                            zin[pr][hi * 64:(hi + 1) * 64,
                                    qb * QB:(qb + 1) * QB],
                            zn,
                        )

            def gather(pr):
                return nc.gpsimd.collective_compute(
                    "AllGather",
                    Alu.bypass,
                    replica_groups=[[0, 1, 2, 3], [4, 5, 6, 7]],
                    ins=[zin[pr].opt()],
                    outs=[zall[pr].opt()],
                )

            # emit in an order that lets the scheduler overlap PE-heavy
            # projection work with the ACT-bound attention phase, and the
            # first AllGather with the second pair's attention
            qk_proj(0)
            v_proj()
            attention(0)
            gather(0)
            qk_proj(1)
            attention(1)
            g1 = gather(1)

            # ---- output projection for this core's 512-row slice ----
            # Split by k-tile parity: even k-tiles only need the pair-0
            # AllGather, so that half runs while the pair-1 collective is
            # still in flight; the odd half + combine follows it.
            wo_sb = persist.tile([P, KT_X, D], BF16)
            nc.sync.dma_start(wo_sb, wo[:].rearrange("(kt p) m -> p kt m", p=P))
            zg_sb = persist.tile([P, KT_X, QB], BF16)
            stage_sb = persist.tile([P, NSLICE // P, D // QB, QB], F32)
            from concourse.bass import ds
            from concourse.tile_rust import add_dep_helper
            # even k-tiles (pair-0 gather) first so the AG#2-gated odd DMAs
            # don't block them on the in-order sync queue
            for kt in [0, 2, 4, 6, 1, 3, 5, 7]:
                zgd = nc.sync.dma_start(
                    zg_sb[:, kt],
                    zall[kt % 2][(kt // 2) * P:(kt // 2 + 1) * P,
                                 ds(qoff_sv, QB)],
                )
                # scheduling-order-only edge: keep these AG-gated DMAs from
                # being placed ahead of attention(1) in the static order,
                # which would serialize attention behind the collective via
                # shared DMA-semaphore counts
                add_dep_helper(zgd.ins, g1.ins, sync=False,
                               reason="zg after gather(1) trigger")
            for mt in range(NSLICE // P):
                for oc in range(D // QB):
                    pse = psA.tile([P, QB], F32, tag="proj", name="pse")
                    for i, kt in enumerate(range(0, KT_X, 2)):
                        nc.tensor.matmul(
                            pse,
                            lhsT=zg_sb[:, kt, mt * P:(mt + 1) * P],
                            rhs=wo_sb[:, kt, oc * QB:(oc + 1) * QB],
                            start=(i == 0),
                            stop=(kt == KT_X - 2),
                        )
                    nc.vector.tensor_copy(stage_sb[:, mt, oc], pse)
            for mt in range(NSLICE // P):
                for oc in range(D // QB):
                    pso = psA.tile([P, QB], F32, tag="proj", name="pso")
                    for i, kt in enumerate(range(1, KT_X, 2)):
                        nc.tensor.matmul(
                            pso,
                            lhsT=zg_sb[:, kt, mt * P:(mt + 1) * P],
                            rhs=wo_sb[:, kt, oc * QB:(oc + 1) * QB],
                            start=(i == 0),
                            stop=(kt == KT_X - 1),
                        )
                    osb = small.tile([P, QB], F32, tag="osb", name="osb")
                    nc.vector.tensor_tensor(
                        osb, pso, stage_sb[:, mt, oc], Alu.add
                    )
                    nc.vector.tensor_tensor(
                        osb, osb, bo_bc[:, oc * QB:(oc + 1) * QB], Alu.add
                    )
                    nc.sync.dma_start(
                        out[mt * P:(mt + 1) * P, oc * QB:(oc + 1) * QB], osb
                    )
    nc.compile()
    return nc


def make_in_maps(inputs):
    x = np.asarray(inputs["inputs"], dtype=np.float32)
    ws = {k: np.asarray(inputs[k], dtype=np.float32) for k in
          ("Wq", "Wk", "Wv", "Wo", "bq", "bk", "bv", "bo")}
    wo_bf = np.ascontiguousarray(ws["Wo"]).astype(BF)
    xT_bf = [np.ascontiguousarray(x[b].T).astype(BF) for b in range(B)]
    in_maps = []
    for c in range(NCORES):
        b, q = c // TP, c % TP
        cols = slice(q * DLOC, (q + 1) * DLOC)
        in_maps.append({
            "xT": xT_bf[b],
            "wq": np.ascontiguousarray(ws["Wq"][:, cols]).astype(BF),
            "wk": np.ascontiguousarray(ws["Wk"][:, cols]).astype(BF),
            "wv": np.ascontiguousarray(ws["Wv"][:, cols]).astype(BF),
            "wo": wo_bf,
            "bq": np.ascontiguousarray(ws["bq"][cols]),
            "bk": np.ascontiguousarray(ws["bk"][cols]),
            "bv": np.ascontiguousarray(ws["bv"][cols]),
            "bo": ws["bo"],
            "qoff": np.array([[q * NSLICE]], dtype=np.uint32),
        })
    return in_maps


def assemble(results):
    outs = [np.asarray(r["out"], dtype=np.float32) for r in results]
    return np.stack(
        [np.concatenate(outs[b * TP:(b + 1) * TP], axis=0) for b in range(B)]
    )


def _ensure_ntff_hook():
    """bass_utils hard-imports antenv.axon_hooks for trace=True; this image
    lacks it.  Shim it and register the ctypes NTFF hook from trn_boot."""
    import types

    if "antenv.axon_hooks" in sys.modules:
        return
    try:
        import antenv.axon_hooks  # noqa: F401
        return
    except ImportError:
        pass
    mod = types.ModuleType("antenv.axon_hooks")
    mod._hook = None
    mod.set_axon_ntff_profile_hook = lambda h: setattr(mod, "_hook", h)
    mod.get_axon_ntff_profile_hook = lambda: mod._hook
    sys.modules["antenv.axon_hooks"] = mod
    try:
        import antenv
        antenv.axon_hooks = mod
    except Exception:
        pass
    try:
        from trn_agent_boot.trn_boot import _ntff_profile_via_ctypes
        hook = _ntff_profile_via_ctypes("/opt/axon/libaxon_pjrt.so")
        if hook is not None:
            mod._hook = hook
    except Exception:
        pass


_cached_nc = None


def kernel(**inputs):
    global _cached_nc
    _ensure_ntff_hook()
    from concourse.bass_utils import run_bass_kernel_spmd

    if _cached_nc is None:
        _cached_nc = build_bass()
    trace = bool(int(os.environ.get("MHA_TRACE", "0")))
    res = run_bass_kernel_spmd(
        _cached_nc, make_in_maps(inputs), core_ids=list(range(NCORES)),
        trace=trace,
    )
    if trace and res.exec_time_ns is not None:
        print(f"HW exec time: {res.exec_time_ns} ns")
        kernel.last_exec_time_ns = res.exec_time_ns
    return assemble(res.results)


# revision 26
# speedup vs baseline: 1.0929x; 1.0929x over previous
"""Multi-head attention (B=2, N=2048, D=1024, H=16) on 8 trn2 NeuronCores.

Sharding: DP2 (batch) x TP4 (head quarters).  Core c handles batch c//4 and
heads [4*(c%4), 4*(c%4)+4).  Per core:
  - QKV projection for its 256 local dims (x^T streamed, weights stationary)
  - causal flash-style attention (no max subtraction: scores/32 are tiny so
    exp is safe; softmax denominator comes from a ones-column fused into the
    z-accumulation matmul)
  - AllToAll of normalized z^T (bf16) within the 4-core batch group: each
    core receives, for every head, exactly its 512-query slice
  - output projection for the core's 512-row slice of the sequence
Host: slices/casts inputs, concatenates the 8 disjoint output slices.
"""

import os
import sys

for _p in ("/opt/trn_rl_repo", "/root/.axon_site/_ro/trn_rl_repo"):
    if os.path.isdir(_p) and _p not in sys.path:
        sys.path.append(_p)

import numpy as np
import ml_dtypes

import concourse.bass as bass
import concourse.mybir as mybir
import concourse.tile as tile
from concourse import bacc

B, N, D, H, HD = 2, 2048, 1024, 16, 64
NCORES, TP = 8, 4
DLOC = D // TP            # 256 local dims (4 heads) per core
P = 128
KT_X = D // P             # 8 contraction tiles for projections
NT = N // P               # 16 n-tiles
QB = 512                  # query block (PSUM bank width in fp32)
NQB = N // QB             # 4
NSLICE = N // TP          # 512 output rows per core
SCALE = 1.0 / 32.0        # 1/sqrt(D)

F32 = mybir.dt.float32
BF16 = mybir.dt.bfloat16
BF = ml_dtypes.bfloat16
Alu = mybir.AluOpType
Act = mybir.ActivationFunctionType


def build_bass():
    nc = bacc.Bacc("TRN2", num_devices=NCORES)

    xT = nc.dram_tensor("xT", [D, N], BF16, kind="ExternalInput")
    wq = nc.dram_tensor("wq", [D, DLOC], BF16, kind="ExternalInput")
    wk = nc.dram_tensor("wk", [D, DLOC], BF16, kind="ExternalInput")
    wv = nc.dram_tensor("wv", [D, DLOC], BF16, kind="ExternalInput")
    wo = nc.dram_tensor("wo", [D, D], BF16, kind="ExternalInput")
    bq = nc.dram_tensor("bq", [DLOC], F32, kind="ExternalInput")
    bk = nc.dram_tensor("bk", [DLOC], F32, kind="ExternalInput")
    bv = nc.dram_tensor("bv", [DLOC], F32, kind="ExternalInput")
    bo = nc.dram_tensor("bo", [D], F32, kind="ExternalInput")
    qoff = nc.dram_tensor("qoff", [1, 1], mybir.dt.uint32, kind="ExternalInput")
    out = nc.dram_tensor("out", [NSLICE, D], F32, kind="ExternalOutput")

    with tile.TileContext(nc) as tc:
        with (
            tc.tile_pool(name="persist", bufs=1) as persist,
            tc.tile_pool(name="wtp", bufs=4) as wtp,
            tc.tile_pool(name="small", bufs=4) as small,
            tc.tile_pool(name="psA", bufs=2, space="PSUM") as psA,
            tc.tile_pool(name="psS", bufs=2, space="PSUM") as psS,
            tc.tile_pool(name="psZ", bufs=4, space="PSUM") as psZ,
            tc.tile_pool(name="dram", bufs=1, space="DRAM") as dram,
        ):
            # ---- constant / input loads (weights first so projection can
            # start as soon as the first xT k-tile lands) ----
            wq_sb = persist.tile([P, KT_X, DLOC], BF16)
            nc.sync.dma_start(wq_sb, wq[:].rearrange("(kt p) m -> p kt m", p=P))
            wk_sb = persist.tile([P, KT_X, DLOC], BF16)
            nc.sync.dma_start(wk_sb, wk[:].rearrange("(kt p) m -> p kt m", p=P))
            wv_sb = persist.tile([P, KT_X, DLOC], BF16)
            nc.sync.dma_start(wv_sb, wv[:].rearrange("(kt p) m -> p kt m", p=P))
            xT_sb = persist.tile([P, KT_X, N], BF16)
            for kt in range(KT_X):
                nc.sync.dma_start(
                    xT_sb[:, kt], xT[kt * P:(kt + 1) * P, :]
                )

            bq_sb = small.tile([P, 2], F32)
            nc.sync.dma_start(bq_sb, bq[:].rearrange("(t p) -> p t", p=P))
            bqs_sb = persist.tile([P, 2], F32)
            nc.vector.tensor_scalar_mul(bqs_sb, bq_sb, SCALE)
            bk_sb = persist.tile([P, 2], F32)
            nc.sync.dma_start(bk_sb, bk[:].rearrange("(t p) -> p t", p=P))

            bv_row = small.tile([1, DLOC], F32)
            nc.sync.dma_start(bv_row, bv[:].rearrange("(a d) -> a d", a=1))
            bv_bc = persist.tile([P, DLOC], F32)
            nc.gpsimd.partition_broadcast(bv_bc, bv_row)

            bo_row = small.tile([1, D], F32)
            nc.sync.dma_start(bo_row, bo[:].rearrange("(a d) -> a d", a=1))
            bo_bc = persist.tile([P, D], F32)
            nc.gpsimd.partition_broadcast(bo_bc, bo_row)

            # staircase causal mask: mask[kk, c] = 1 iff c >= kk + 384;
            # view [:, 384-o : 384-o+width] gives "keep iff qq >= kk + o"
            mask_sb = persist.tile([P, QB + 384], BF16)
            nc.gpsimd.memset(mask_sb, 1.0)
            nc.gpsimd.affine_select(
                out=mask_sb,
                in_=mask_sb,
                compare_op=Alu.is_ge,
                fill=0.0,
                base=-384,
                pattern=[[1, QB + 384]],
                channel_multiplier=-1,
            )

            # ---- QKV projection helpers ----
            qT_sb = persist.tile([P, 2, N], BF16)
            kT_sb = persist.tile([P, 2, N], BF16)
            v_sb = persist.tile([P, NT, 4 * 65], BF16)
            nc.vector.memset(v_sb, 1.0)  # preset ones columns for denominators
            ones_sb = persist.tile([1, 64], F32)
            nc.vector.memset(ones_sb, 1.0)

            def qk_proj(mt):
                for qc in range(NQB):
                    psq = psA.tile([P, QB], F32, tag="proj", name="psq")
                    for kt in range(KT_X):
                        nc.tensor.matmul(
                            psq,
                            lhsT=wq_sb[:, kt, mt * P:(mt + 1) * P],
                            rhs=xT_sb[:, kt, qc * QB:(qc + 1) * QB],
                            start=(kt == 0),
                            stop=(kt == KT_X - 1),
                        )
                    nc.vector.tensor_scalar(
                        qT_sb[:, mt, qc * QB:(qc + 1) * QB],
                        psq,
                        SCALE,
                        bqs_sb[:, mt:mt + 1],
                        Alu.mult,
                        Alu.add,
                    )
                    psk = psA.tile([P, QB], F32, tag="proj", name="psk")
                    for kt in range(KT_X):
                        nc.tensor.matmul(
                            psk,
                            lhsT=wk_sb[:, kt, mt * P:(mt + 1) * P],
                            rhs=xT_sb[:, kt, qc * QB:(qc + 1) * QB],
                            start=(kt == 0),
                            stop=(kt == KT_X - 1),
                        )
                    nc.vector.tensor_scalar_add(
                        kT_sb[:, mt, qc * QB:(qc + 1) * QB],
                        psk,
                        bk_sb[:, mt:mt + 1],
                    )

            def v_proj():
                for nt in range(NT):
                    psv_full = psA.tile([P, QB], F32, tag="proj", name="psv")
                    psv = psv_full[:, :DLOC]
                    for kt in range(KT_X):
                        nc.tensor.matmul(
                            psv,
                            lhsT=xT_sb[:, kt, nt * P:(nt + 1) * P],
                            rhs=wv_sb[:, kt, :],
                            start=(kt == 0),
                            stop=(kt == KT_X - 1),
                        )
                    nc.vector.tensor_tensor(
                        v_sb[:, nt].rearrange("p (h x) -> p h x", x=65)[:, :, 0:64],
                        psv.rearrange("p (h x) -> p h x", x=64),
                        bv_bc.rearrange("p (h x) -> p h x", x=64),
                        Alu.add,
                    )

            # per-core dynamic sequence offset for the output projection
            qoff_sb = small.tile([1, 1], mybir.dt.uint32)
            nc.sync.dma_start(qoff_sb, qoff[:])
            qregs = nc.alloc_registers()
            nc.regs_load(qregs, qoff_sb[0:1, 0:1])
            qoff_sv = nc.snap(qregs, donate=True)

            # one AllGather per head pair so the first overlaps the second
            # pair's attention.  zallX rows: rank r block = global dims
            # [r*256 + pair*128, +128).
            zin = [dram.tile([P, N], BF16, name=f"zin{pr}") for pr in range(2)]
            zall = [dram.tile([TP * P, N], BF16, name=f"zall{pr}")
                    for pr in range(2)]

            # ---- attention for one head pair ----
            def attention(pr):
                for qb in range(NQB):
                    kt_max = (qb + 1) * 4
                    zps = [psZ.tile([65, QB], F32, tag="z", name=f"zp{hi}")
                           for hi in range(2)]
                    # software-pipelined emission: the z matmuls of k-tile
                    # kt-1 are emitted after the score pair of k-tile kt, so
                    # the two score matmuls sit adjacent in the PE stream and
                    # run concurrently on disjoint row groups (contraction
                    # partitions 0-63 / 64-127)
                    pending_z = []
                    for kt in range(kt_max):
                        diag = kt >= qb * 4
                        o = kt * P - qb * QB if diag else 0
                        sps = []
                        for hi in range(2):
                            spf = psS.tile([P, QB], F32, tag="score", name="spf")
                            sp = spf[:, o:QB]
                            nc.tensor.matmul(
                                sp,
                                lhsT=kT_sb[hi * 64:(hi + 1) * 64, pr,
                                           kt * P:(kt + 1) * P],
                                rhs=qT_sb[hi * 64:(hi + 1) * 64, pr,
                                          qb * QB + o:(qb + 1) * QB],
                                start=True,
                                stop=True,
                                tile_position=(hi * 64, 0),
                            )
                            sps.append(sp)
                        for args in pending_z:
                            nc.tensor.matmul(**args)
                        pending_z = []
                        for hi in range(2):
                            wt = wtp.tile([P, QB], BF16, tag="wt", name="wt")
                            nc.scalar.activation(wt[:, o:QB], sps[hi], Act.Exp)
                            if diag:
                                # only the o..o+128 strip straddles the diagonal
                                nc.vector.tensor_tensor(
                                    wt[:, o:o + P],
                                    wt[:, o:o + P],
                                    mask_sb[:, 384:384 + P],
                                    Alu.mult,
                                )
                            pending_z.append(dict(
                                out=zps[hi][:, o:QB],
                                lhsT=v_sb[:, kt,
                                          (2 * pr + hi) * 65:(2 * pr + hi + 1) * 65],
                                rhs=wt[:, o:QB],
                                start=(kt == 0),
                                stop=(kt == kt_max - 1),
                                skip_group_check=True,
                            ))
                    for args in pending_z:
                        nc.tensor.matmul(**args)
                    for hi in range(2):
                        # denominator -> SBUF (approx_fast misbehaves on a
                        # PSUM source), reciprocal, gpsimd partition
                        # broadcast, normalize.  No PE in this chain, and
                        # the zin write rides the gpsimd queue so the sync
                        # queue's post-collective DMAs can't block it.
                        den = small.tile([1, QB], F32, tag="den", name="den")
                        nc.vector.tensor_copy(den, zps[hi][64:65, :])
                        recip = small.tile([1, QB], F32, tag="recip", name="recip")
                        nc.vector.reciprocal_approx_fast(recip, den)
                        rb = small.tile([64, QB], F32, tag="rb", name="rb")
                        nc.gpsimd.partition_broadcast(rb, recip)
                        zn = small.tile([64, QB], BF16, tag="zn", name="zn")
                        nc.vector.tensor_tensor(zn, zps[hi][0:64, :], rb, Alu.mult)
                        nc.gpsimd.dma_start(
                            zin[pr][hi * 64:(hi + 1) * 64,
                                    qb * QB:(qb + 1) * QB],
                            zn,
                        )

            def gather(pr):
                return nc.gpsimd.collective_compute(
                    "AllGather",
                    Alu.bypass,
                    replica_groups=[[0, 1, 2, 3], [4, 5, 6, 7]],
                    ins=[zin[pr].opt()],
                    outs=[zall[pr].opt()],
                )

            # emit in an order that lets the scheduler overlap PE-heavy
            # projection work with the ACT-bound attention phase, and the
            # first AllGather with the second pair's attention
            qk_proj(0)
            v_proj()
            attention(0)
            gather(0)
            qk_proj(1)
            attention(1)
            g1 = gather(1)

            # ---- output projection for this core's 512-row slice ----
            # Split by k-tile parity: even k-tiles only need the pair-0
            # AllGather, so that half runs while the pair-1 collective is
            # still in flight; the odd half + combine follows it.
            wo_sb = persist.tile([P, KT_X, D], BF16)
            nc.sync.dma_start(wo_sb, wo[:].rearrange("(kt p) m -> p kt m", p=P))
            zg_sb = persist.tile([P, KT_X, QB], BF16)
            stage_sb = persist.tile([P, NSLICE // P, D // QB, QB], F32)
            from concourse.bass import ds
            from concourse.tile_rust import add_dep_helper
            # even k-tiles (pair-0 gather) first so the AG#2-gated odd DMAs
            # don't block them on the in-order sync queue
            for kt in [0, 2, 4, 6, 1, 3, 5, 7]:
                zgd = nc.sync.dma_start(
                    zg_sb[:, kt],
                    zall[kt % 2][(kt // 2) * P:(kt // 2 + 1) * P,
                                 ds(qoff_sv, QB)],
                )
                # scheduling-order-only edge: keep these AG-gated DMAs from
                # being placed ahead of attention(1) in the static order,
                # which would serialize attention behind the collective via
                # shared DMA-semaphore counts
                add_dep_helper(zgd.ins, g1.ins, sync=False,
                               reason="zg after gather(1) trigger")
            for mt in range(NSLICE // P):
                for oc in range(D // QB):
                    pse = psA.tile([P, QB], F32, tag="proj", name="pse")
                    for i, kt in enumerate(range(0, KT_X, 2)):
                        nc.tensor.matmul(
                            pse,
                            lhsT=zg_sb[:, kt, mt * P:(mt + 1) * P],
                            rhs=wo_sb[:, kt, oc * QB:(oc + 1) * QB],
                            start=(i == 0),
                            stop=(kt == KT_X - 2),
                        )
                    nc.vector.tensor_copy(stage_sb[:, mt, oc], pse)
            for mt in range(NSLICE // P):
                for oc in range(D // QB):
                    pso = psA.tile([P, QB], F32, tag="proj", name="pso")
                    for i, kt in enumerate(range(1, KT_X, 2)):
                        nc.tensor.matmul(
                            pso,
                            lhsT=zg_sb[:, kt, mt * P:(mt + 1) * P],
                            rhs=wo_sb[:, kt, oc * QB:(oc + 1) * QB],
                            start=(i == 0),
                            stop=(kt == KT_X - 1),
                        )
                    osb = small.tile([P, QB], F32, tag="osb", name="osb")
                    nc.vector.tensor_tensor(
                        osb, pso, stage_sb[:, mt, oc], Alu.add
                    )
                    nc.vector.tensor_tensor(
                        osb, osb, bo_bc[:, oc * QB:(oc + 1) * QB], Alu.add
                    )
                    nc.sync.dma_start(
                        out[mt * P:(mt + 1) * P, oc * QB:(oc + 1) * QB], osb
                    )
    nc.compile()
    return nc


def make_in_maps(inputs):
    x = np.asarray(inputs["inputs"], dtype=np.float32)
    ws = {k: np.asarray(inputs[k], dtype=np.float32) for k in
          ("Wq", "Wk", "Wv", "Wo", "bq", "bk", "bv", "bo")}
    wo_bf = np.ascontiguousarray(ws["Wo"]).astype(BF)
    xT_bf = [np.ascontiguousarray(x[b].T).astype(BF) for b in range(B)]
    in_maps = []
    for c in range(NCORES):
        b, q = c // TP, c % TP
        cols = slice(q * DLOC, (q + 1) * DLOC)
        in_maps.append({
            "xT": xT_bf[b],
            "wq": np.ascontiguousarray(ws["Wq"][:, cols]).astype(BF),
            "wk": np.ascontiguousarray(ws["Wk"][:, cols]).astype(BF),
            "wv": np.ascontiguousarray(ws["Wv"][:, cols]).astype(BF),
            "wo": wo_bf,
            "bq": np.ascontiguousarray(ws["bq"][cols]),
            "bk": np.ascontiguousarray(ws["bk"][cols]),
            "bv": np.ascontiguousarray(ws["bv"][cols]),
            "bo": ws["bo"],
            "qoff": np.array([[q * NSLICE]], dtype=np.uint32),
        })
    return in_maps


def assemble(results):
    outs = [np.asarray(r["out"], dtype=np.float32) for r in results]
    return np.stack(
        [np.concatenate(outs[b * TP:(b + 1) * TP], axis=0) for b in range(B)]
    )


def _ensure_ntff_hook():
    """bass_utils hard-imports antenv.axon_hooks for trace=True; this image
    lacks it.  Shim it and register the ctypes NTFF hook from trn_boot."""
    import types

    if "antenv.axon_hooks" in sys.modules:
        return
    try:
        import antenv.axon_hooks  # noqa: F401
        return
    except ImportError:
        pass
    mod = types.ModuleType("antenv.axon_hooks")
    mod._hook = None
    mod.set_axon_ntff_profile_hook = lambda h: setattr(mod, "_hook", h)
    mod.get_axon_ntff_profile_hook = lambda: mod._hook
    sys.modules["antenv.axon_hooks"] = mod
    try:
        import antenv
        antenv.axon_hooks = mod
    except Exception:
        pass
    try:
        from trn_agent_boot.trn_boot import _ntff_profile_via_ctypes
        hook = _ntff_profile_via_ctypes("/opt/axon/libaxon_pjrt.so")
        if hook is not None:
            mod._hook = hook
    except Exception:
        pass


_cached_nc = None


def kernel(**inputs):
    global _cached_nc
    _ensure_ntff_hook()
    from concourse.bass_utils import run_bass_kernel_spmd

    if _cached_nc is None:
        _cached_nc = build_bass()
    trace = bool(int(os.environ.get("MHA_TRACE", "0")))
    res = run_bass_kernel_spmd(
        _cached_nc, make_in_maps(inputs), core_ids=list(range(NCORES)),
        trace=trace,
    )
    if trace and res.exec_time_ns is not None:
        print(f"HW exec time: {res.exec_time_ns} ns")
        kernel.last_exec_time_ns = res.exec_time_ns
    return assemble(res.results)


# revision 27
# speedup vs baseline: 1.1451x; 1.0477x over previous
"""Multi-head attention (B=2, N=2048, D=1024, H=16) on 8 trn2 NeuronCores.

Sharding: DP2 (batch) x TP4 (head quarters).  Core c handles batch c//4 and
heads [4*(c%4), 4*(c%4)+4).  Per core:
  - QKV projection for its 256 local dims (x^T streamed, weights stationary)
  - causal flash-style attention (no max subtraction: scores/32 are tiny so
    exp is safe; softmax denominator comes from a ones-column fused into the
    z-accumulation matmul)
  - AllToAll of normalized z^T (bf16) within the 4-core batch group: each
    core receives, for every head, exactly its 512-query slice
  - output projection for the core's 512-row slice of the sequence
Host: slices/casts inputs, concatenates the 8 disjoint output slices.
"""

import os
import sys

for _p in ("/opt/trn_rl_repo", "/root/.axon_site/_ro/trn_rl_repo"):
    if os.path.isdir(_p) and _p not in sys.path:
        sys.path.append(_p)

import numpy as np
import ml_dtypes

import concourse.bass as bass
import concourse.mybir as mybir
import concourse.tile as tile
from concourse import bacc

B, N, D, H, HD = 2, 2048, 1024, 16, 64
NCORES, TP = 8, 4
DLOC = D // TP            # 256 local dims (4 heads) per core
P = 128
KT_X = D // P             # 8 contraction tiles for projections
NT = N // P               # 16 n-tiles
QB = 512                  # query block (PSUM bank width in fp32)
NQB = N // QB             # 4
NSLICE = N // TP          # 512 output rows per core
SCALE = 1.0 / 32.0        # 1/sqrt(D)

F32 = mybir.dt.float32
BF16 = mybir.dt.bfloat16
BF = ml_dtypes.bfloat16
Alu = mybir.AluOpType
Act = mybir.ActivationFunctionType


def build_bass():
    nc = bacc.Bacc("TRN2", num_devices=NCORES)

    xT = nc.dram_tensor("xT", [D, N], BF16, kind="ExternalInput")
    wq = nc.dram_tensor("wq", [D, DLOC], BF16, kind="ExternalInput")
    wk = nc.dram_tensor("wk", [D, DLOC], BF16, kind="ExternalInput")
    wv = nc.dram_tensor("wv", [D, DLOC], BF16, kind="ExternalInput")
    wo = nc.dram_tensor("wo", [D, D], BF16, kind="ExternalInput")
    bq = nc.dram_tensor("bq", [DLOC], F32, kind="ExternalInput")
    bk = nc.dram_tensor("bk", [DLOC], F32, kind="ExternalInput")
    bv = nc.dram_tensor("bv", [DLOC], F32, kind="ExternalInput")
    bo = nc.dram_tensor("bo", [D], F32, kind="ExternalInput")
    qoff = nc.dram_tensor("qoff", [1, 1], mybir.dt.uint32, kind="ExternalInput")
    out = nc.dram_tensor("out", [NSLICE, D], F32, kind="ExternalOutput")

    with tile.TileContext(nc) as tc:
        with (
            tc.tile_pool(name="persist", bufs=1) as persist,
            tc.tile_pool(name="wtp", bufs=4) as wtp,
            tc.tile_pool(name="small", bufs=4) as small,
            tc.tile_pool(name="psA", bufs=1, space="PSUM") as psA,
            tc.tile_pool(name="psS", bufs=4, space="PSUM") as psS,
            tc.tile_pool(name="psZ", bufs=3, space="PSUM") as psZ,
            tc.tile_pool(name="dram", bufs=1, space="DRAM") as dram,
        ):
            # ---- constant / input loads (weights first so projection can
            # start as soon as the first xT k-tile lands) ----
            wq_sb = persist.tile([P, KT_X, DLOC], BF16)
            nc.sync.dma_start(wq_sb, wq[:].rearrange("(kt p) m -> p kt m", p=P))
            wk_sb = persist.tile([P, KT_X, DLOC], BF16)
            nc.sync.dma_start(wk_sb, wk[:].rearrange("(kt p) m -> p kt m", p=P))
            wv_sb = persist.tile([P, KT_X, DLOC], BF16)
            nc.sync.dma_start(wv_sb, wv[:].rearrange("(kt p) m -> p kt m", p=P))
            xT_sb = persist.tile([P, KT_X, N], BF16)
            for kt in range(KT_X):
                nc.sync.dma_start(
                    xT_sb[:, kt], xT[kt * P:(kt + 1) * P, :]
                )

            bq_sb = small.tile([P, 2], F32)
            nc.sync.dma_start(bq_sb, bq[:].rearrange("(t p) -> p t", p=P))
            bqs_sb = persist.tile([P, 2], F32)
            nc.vector.tensor_scalar_mul(bqs_sb, bq_sb, SCALE)
            bk_sb = persist.tile([P, 2], F32)
            nc.sync.dma_start(bk_sb, bk[:].rearrange("(t p) -> p t", p=P))

            bv_row = small.tile([1, DLOC], F32)
            nc.sync.dma_start(bv_row, bv[:].rearrange("(a d) -> a d", a=1))
            bv_bc = persist.tile([P, DLOC], F32)
            nc.gpsimd.partition_broadcast(bv_bc, bv_row)

            bo_row = small.tile([1, D], F32)
            nc.sync.dma_start(bo_row, bo[:].rearrange("(a d) -> a d", a=1))
            bo_bc = persist.tile([P, D], F32)
            nc.gpsimd.partition_broadcast(bo_bc, bo_row)

            # staircase causal mask: mask[kk, c] = 1 iff c >= kk + 384;
            # view [:, 384-o : 384-o+width] gives "keep iff qq >= kk + o"
            mask_sb = persist.tile([P, QB + 384], BF16)
            nc.gpsimd.memset(mask_sb, 1.0)
            nc.gpsimd.affine_select(
                out=mask_sb,
                in_=mask_sb,
                compare_op=Alu.is_ge,
                fill=0.0,
                base=-384,
                pattern=[[1, QB + 384]],
                channel_multiplier=-1,
            )

            # ---- QKV projection helpers ----
            qT_sb = persist.tile([P, 2, N], BF16)
            kT_sb = persist.tile([P, 2, N], BF16)
            v_sb = persist.tile([P, NT, 4 * 65], BF16)
            nc.vector.memset(v_sb, 1.0)  # preset ones columns for denominators
            ones_sb = persist.tile([1, 64], F32)
            nc.vector.memset(ones_sb, 1.0)

            def qk_proj(mt):
                for qc in range(NQB):
                    psq = psA.tile([P, QB], F32, tag="proj", name="psq")
                    for kt in range(KT_X):
                        nc.tensor.matmul(
                            psq,
                            lhsT=wq_sb[:, kt, mt * P:(mt + 1) * P],
                            rhs=xT_sb[:, kt, qc * QB:(qc + 1) * QB],
                            start=(kt == 0),
                            stop=(kt == KT_X - 1),
                        )
                    nc.vector.tensor_scalar(
                        qT_sb[:, mt, qc * QB:(qc + 1) * QB],
                        psq,
                        SCALE,
                        bqs_sb[:, mt:mt + 1],
                        Alu.mult,
                        Alu.add,
                    )
                    psk = psA.tile([P, QB], F32, tag="proj", name="psk")
                    for kt in range(KT_X):
                        nc.tensor.matmul(
                            psk,
                            lhsT=wk_sb[:, kt, mt * P:(mt + 1) * P],
                            rhs=xT_sb[:, kt, qc * QB:(qc + 1) * QB],
                            start=(kt == 0),
                            stop=(kt == KT_X - 1),
                        )
                    nc.vector.tensor_scalar_add(
                        kT_sb[:, mt, qc * QB:(qc + 1) * QB],
                        psk,
                        bk_sb[:, mt:mt + 1],
                    )

            def v_proj():
                for nt in range(NT):
                    psv_full = psA.tile([P, QB], F32, tag="proj", name="psv")
                    psv = psv_full[:, :DLOC]
                    for kt in range(KT_X):
                        nc.tensor.matmul(
                            psv,
                            lhsT=xT_sb[:, kt, nt * P:(nt + 1) * P],
                            rhs=wv_sb[:, kt, :],
                            start=(kt == 0),
                            stop=(kt == KT_X - 1),
                        )
                    nc.vector.tensor_tensor(
                        v_sb[:, nt].rearrange("p (h x) -> p h x", x=65)[:, :, 0:64],
                        psv.rearrange("p (h x) -> p h x", x=64),
                        bv_bc.rearrange("p (h x) -> p h x", x=64),
                        Alu.add,
                    )

            # per-core dynamic sequence offset for the output projection
            qoff_sb = small.tile([1, 1], mybir.dt.uint32)
            nc.sync.dma_start(qoff_sb, qoff[:])
            qregs = nc.alloc_registers()
            nc.regs_load(qregs, qoff_sb[0:1, 0:1])
            qoff_sv = nc.snap(qregs, donate=True)

            # one AllGather per head pair so the first overlaps the second
            # pair's attention.  zallX rows: rank r block = global dims
            # [r*256 + pair*128, +128).
            zin = [dram.tile([P, N], BF16, name=f"zin{pr}") for pr in range(2)]
            zall = [dram.tile([TP * P, N], BF16, name=f"zall{pr}")
                    for pr in range(2)]

            # ---- attention for one head pair ----
            def attention(pr):
                for qb in range(NQB):
                    kt_max = (qb + 1) * 4
                    zps = [psZ.tile([65, QB], F32, tag="z", name=f"zp{hi}")
                           for hi in range(2)]
                    # software-pipelined emission: the z matmuls of k-tile
                    # kt-1 are emitted after the score pair of k-tile kt, so
                    # the two score matmuls sit adjacent in the PE stream and
                    # run concurrently on disjoint row groups (contraction
                    # partitions 0-63 / 64-127)
                    pending_z = []
                    for kt in range(kt_max):
                        diag = kt >= qb * 4
                        o = kt * P - qb * QB if diag else 0
                        sps = []
                        for hi in range(2):
                            spf = psS.tile([P, QB], F32, tag="score", name="spf")
                            sp = spf[:, o:QB]
                            nc.tensor.matmul(
                                sp,
                                lhsT=kT_sb[hi * 64:(hi + 1) * 64, pr,
                                           kt * P:(kt + 1) * P],
                                rhs=qT_sb[hi * 64:(hi + 1) * 64, pr,
                                          qb * QB + o:(qb + 1) * QB],
                                start=True,
                                stop=True,
                                tile_position=(hi * 64, 0),
                            )
                            sps.append(sp)
                        for args in pending_z:
                            nc.tensor.matmul(**args)
                        pending_z = []
                        for hi in range(2):
                            wt = wtp.tile([P, QB], BF16, tag="wt", name="wt")
                            nc.scalar.activation(wt[:, o:QB], sps[hi], Act.Exp)
                            if diag:
                                # only the o..o+128 strip straddles the diagonal
                                nc.vector.tensor_tensor(
                                    wt[:, o:o + P],
                                    wt[:, o:o + P],
                                    mask_sb[:, 384:384 + P],
                                    Alu.mult,
                                )
                            pending_z.append(dict(
                                out=zps[hi][:, o:QB],
                                lhsT=v_sb[:, kt,
                                          (2 * pr + hi) * 65:(2 * pr + hi + 1) * 65],
                                rhs=wt[:, o:QB],
                                start=(kt == 0),
                                stop=(kt == kt_max - 1),
                                skip_group_check=True,
                            ))
                    for args in pending_z:
                        nc.tensor.matmul(**args)
                    for hi in range(2):
                        # denominator -> SBUF (approx_fast misbehaves on a
                        # PSUM source), reciprocal, gpsimd partition
                        # broadcast, normalize.  No PE in this chain, and
                        # the zin write rides the gpsimd queue so the sync
                        # queue's post-collective DMAs can't block it.
                        den = small.tile([1, QB], F32, tag="den", name="den")
                        nc.vector.tensor_copy(den, zps[hi][64:65, :])
                        recip = small.tile([1, QB], F32, tag="recip", name="recip")
                        nc.vector.reciprocal_approx_fast(recip, den)
                        rb = small.tile([64, QB], F32, tag="rb", name="rb")
                        nc.gpsimd.partition_broadcast(rb, recip)
                        zn = small.tile([64, QB], BF16, tag="zn", name="zn")
                        nc.vector.tensor_tensor(zn, zps[hi][0:64, :], rb, Alu.mult)
                        nc.gpsimd.dma_start(
                            zin[pr][hi * 64:(hi + 1) * 64,
                                    qb * QB:(qb + 1) * QB],
                            zn,
                        )

            def gather(pr):
                return nc.gpsimd.collective_compute(
                    "AllGather",
                    Alu.bypass,
                    replica_groups=[[0, 1, 2, 3], [4, 5, 6, 7]],
                    ins=[zin[pr].opt()],
                    outs=[zall[pr].opt()],
                )

            # emit in an order that lets the scheduler overlap PE-heavy
            # projection work with the ACT-bound attention phase, and the
            # first AllGather with the second pair's attention
            qk_proj(0)
            v_proj()
            attention(0)
            gather(0)
            qk_proj(1)
            attention(1)
            g1 = gather(1)

            # ---- output projection for this core's 512-row slice ----
            # Split by k-tile parity: even k-tiles only need the pair-0
            # AllGather, so that half runs while the pair-1 collective is
            # still in flight; the odd half + combine follows it.
            wo_sb = persist.tile([P, KT_X, D], BF16)
            nc.sync.dma_start(wo_sb, wo[:].rearrange("(kt p) m -> p kt m", p=P))
            zg_sb = persist.tile([P, KT_X, QB], BF16)
            stage_sb = persist.tile([P, NSLICE // P, D // QB, QB], F32)
            from concourse.bass import ds
            from concourse.tile_rust import add_dep_helper
            # even k-tiles (pair-0 gather) first so the AG#2-gated odd DMAs
            # don't block them on the in-order sync queue
            for kt in [0, 2, 4, 6, 1, 3, 5, 7]:
                zgd = nc.sync.dma_start(
                    zg_sb[:, kt],
                    zall[kt % 2][(kt // 2) * P:(kt // 2 + 1) * P,
                                 ds(qoff_sv, QB)],
                )
                # scheduling-order-only edge: keep these AG-gated DMAs from
                # being placed ahead of attention(1) in the static order,
                # which would serialize attention behind the collective via
                # shared DMA-semaphore counts
                add_dep_helper(zgd.ins, g1.ins, sync=False,
                               reason="zg after gather(1) trigger")
            for mt in range(NSLICE // P):
                for oc in range(D // QB):
                    pse = psA.tile([P, QB], F32, tag="proj", name="pse")
                    for i, kt in enumerate(range(0, KT_X, 2)):
                        nc.tensor.matmul(
                            pse,
                            lhsT=zg_sb[:, kt, mt * P:(mt + 1) * P],
                            rhs=wo_sb[:, kt, oc * QB:(oc + 1) * QB],
                            start=(i == 0),
                            stop=(kt == KT_X - 2),
                        )
                    nc.vector.tensor_copy(stage_sb[:, mt, oc], pse)
            for mt in range(NSLICE // P):
                for oc in range(D // QB):
                    pso = psA.tile([P, QB], F32, tag="proj", name="pso")
                    for i, kt in enumerate(range(1, KT_X, 2)):
                        nc.tensor.matmul(
                            pso,
                            lhsT=zg_sb[:, kt, mt * P:(mt + 1) * P],
                            rhs=wo_sb[:, kt, oc * QB:(oc + 1) * QB],
                            start=(i == 0),
                            stop=(kt == KT_X - 1),
                        )
                    osb = small.tile([P, QB], F32, tag="osb", name="osb")
                    nc.vector.tensor_tensor(
                        osb, pso, stage_sb[:, mt, oc], Alu.add
                    )
                    nc.vector.tensor_tensor(
                        osb, osb, bo_bc[:, oc * QB:(oc + 1) * QB], Alu.add
                    )
                    nc.sync.dma_start(
                        out[mt * P:(mt + 1) * P, oc * QB:(oc + 1) * QB], osb
                    )
    nc.compile()
    return nc


def make_in_maps(inputs):
    x = np.asarray(inputs["inputs"], dtype=np.float32)
    ws = {k: np.asarray(inputs[k], dtype=np.float32) for k in
          ("Wq", "Wk", "Wv", "Wo", "bq", "bk", "bv", "bo")}
    wo_bf = np.ascontiguousarray(ws["Wo"]).astype(BF)
    xT_bf = [np.ascontiguousarray(x[b].T).astype(BF) for b in range(B)]
    in_maps = []
    for c in range(NCORES):
        b, q = c // TP, c % TP
        cols = slice(q * DLOC, (q + 1) * DLOC)
        in_maps.append({
            "xT": xT_bf[b],
            "wq": np.ascontiguousarray(ws["Wq"][:, cols]).astype(BF),
            "wk": np.ascontiguousarray(ws["Wk"][:, cols]).astype(BF),
            "wv": np.ascontiguousarray(ws["Wv"][:, cols]).astype(BF),
            "wo": wo_bf,
            "bq": np.ascontiguousarray(ws["bq"][cols]),
            "bk": np.ascontiguousarray(ws["bk"][cols]),
            "bv": np.ascontiguousarray(ws["bv"][cols]),
            "bo": ws["bo"],
            "qoff": np.array([[q * NSLICE]], dtype=np.uint32),
        })
    return in_maps


def assemble(results):
    outs = [np.asarray(r["out"], dtype=np.float32) for r in results]
    return np.stack(
        [np.concatenate(outs[b * TP:(b + 1) * TP], axis=0) for b in range(B)]
    )


def _ensure_ntff_hook():
    """bass_utils hard-imports antenv.axon_hooks for trace=True; this image
    lacks it.  Shim it and register the ctypes NTFF hook from trn_boot."""
    import types

    if "antenv.axon_hooks" in sys.modules:
        return
    try:
        import antenv.axon_hooks  # noqa: F401
        return
    except ImportError:
        pass
    mod = types.ModuleType("antenv.axon_hooks")
    mod._hook = None
    mod.set_axon_ntff_profile_hook = lambda h: setattr(mod, "_hook", h)
    mod.get_axon_ntff_profile_hook = lambda: mod._hook
    sys.modules["antenv.axon_hooks"] = mod
    try:
        import antenv
        antenv.axon_hooks = mod
    except Exception:
        pass
    try:
        from trn_agent_boot.trn_boot import _ntff_profile_via_ctypes
        hook = _ntff_profile_via_ctypes("/opt/axon/libaxon_pjrt.so")
        if hook is not None:
            mod._hook = hook
    except Exception:
        pass


_cached_nc = None


def kernel(**inputs):
    global _cached_nc
    _ensure_ntff_hook()
    from concourse.bass_utils import run_bass_kernel_spmd

    if _cached_nc is None:
        _cached_nc = build_bass()
    trace = bool(int(os.environ.get("MHA_TRACE", "0")))
    res = run_bass_kernel_spmd(
        _cached_nc, make_in_maps(inputs), core_ids=list(range(NCORES)),
        trace=trace,
    )
    if trace and res.exec_time_ns is not None:
        print(f"HW exec time: {res.exec_time_ns} ns")
        kernel.last_exec_time_ns = res.exec_time_ns
    return assemble(res.results)


# revision 32
# speedup vs baseline: 1.2123x; 1.0588x over previous
"""Multi-head attention (B=2, N=2048, D=1024, H=16) on 8 trn2 NeuronCores.

Sharding: DP2 (batch) x TP4 (head quarters).  Core c handles batch c//4 and
heads [4*(c%4), 4*(c%4)+4).  Per core:
  - QKV projection for its 256 local dims (x^T streamed, weights stationary)
  - causal flash-style attention (no max subtraction: scores/32 are tiny so
    exp is safe; softmax denominator comes from a ones-column fused into the
    z-accumulation matmul)
  - AllToAll of normalized z^T (bf16) within the 4-core batch group: each
    core receives, for every head, exactly its 512-query slice
  - output projection for the core's 512-row slice of the sequence
Host: slices/casts inputs, concatenates the 8 disjoint output slices.
"""

import os
import sys

for _p in ("/opt/trn_rl_repo", "/root/.axon_site/_ro/trn_rl_repo"):
    if os.path.isdir(_p) and _p not in sys.path:
        sys.path.append(_p)

import numpy as np
import ml_dtypes

import concourse.bass as bass
import concourse.mybir as mybir
import concourse.tile as tile
from concourse import bacc

B, N, D, H, HD = 2, 2048, 1024, 16, 64
NCORES, TP = 8, 4
DLOC = D // TP            # 256 local dims (4 heads) per core
P = 128
KT_X = D // P             # 8 contraction tiles for projections
NT = N // P               # 16 n-tiles
QB = 512                  # query block (PSUM bank width in fp32)
NQB = N // QB             # 4
NSLICE = N // TP          # 512 output rows per core
SCALE = 1.0 / 32.0        # 1/sqrt(D)

F32 = mybir.dt.float32
BF16 = mybir.dt.bfloat16
BF = ml_dtypes.bfloat16
Alu = mybir.AluOpType
Act = mybir.ActivationFunctionType


def build_bass():
    nc = bacc.Bacc("TRN2", num_devices=NCORES)

    xT = nc.dram_tensor("xT", [D, N], BF16, kind="ExternalInput")
    wq = nc.dram_tensor("wq", [D, DLOC], BF16, kind="ExternalInput")
    wk = nc.dram_tensor("wk", [D, DLOC], BF16, kind="ExternalInput")
    wv = nc.dram_tensor("wv", [D, DLOC], BF16, kind="ExternalInput")
    wo = nc.dram_tensor("wo", [D, D], BF16, kind="ExternalInput")
    bq = nc.dram_tensor("bq", [DLOC], F32, kind="ExternalInput")
    bk = nc.dram_tensor("bk", [DLOC], F32, kind="ExternalInput")
    bv = nc.dram_tensor("bv", [DLOC], F32, kind="ExternalInput")
    bo = nc.dram_tensor("bo", [D], F32, kind="ExternalInput")
    qoff = nc.dram_tensor("qoff", [1, 1], mybir.dt.uint32, kind="ExternalInput")
    out = nc.dram_tensor("out", [NSLICE, D], F32, kind="ExternalOutput")

    with tile.TileContext(nc) as tc:
        with (
            tc.tile_pool(name="persist", bufs=1) as persist,
            tc.tile_pool(name="wtp", bufs=4) as wtp,
            tc.tile_pool(name="small", bufs=4) as small,
            tc.tile_pool(name="psA", bufs=1, space="PSUM") as psA,
            tc.tile_pool(name="psS", bufs=4, space="PSUM") as psS,
            tc.tile_pool(name="psZ", bufs=3, space="PSUM") as psZ,
            tc.tile_pool(name="dram", bufs=1, space="DRAM") as dram,
        ):
            # ---- constant / input loads (weights first so projection can
            # start as soon as the first xT k-tile lands) ----
            wq_sb = persist.tile([P, KT_X, DLOC], BF16)
            nc.sync.dma_start(wq_sb, wq[:].rearrange("(kt p) m -> p kt m", p=P))
            wk_sb = persist.tile([P, KT_X, DLOC], BF16)
            nc.sync.dma_start(wk_sb, wk[:].rearrange("(kt p) m -> p kt m", p=P))
            wv_sb = persist.tile([P, KT_X, DLOC], BF16)
            nc.sync.dma_start(wv_sb, wv[:].rearrange("(kt p) m -> p kt m", p=P))
            xT_sb = persist.tile([P, KT_X, N], BF16)
            for kt in range(KT_X):
                nc.sync.dma_start(
                    xT_sb[:, kt], xT[kt * P:(kt + 1) * P, :]
                )

            bq_sb = small.tile([P, 2], F32)
            nc.sync.dma_start(bq_sb, bq[:].rearrange("(t p) -> p t", p=P))
            bqs_sb = persist.tile([P, 2], F32)
            nc.vector.tensor_scalar_mul(bqs_sb, bq_sb, SCALE)
            bk_sb = persist.tile([P, 2], F32)
            nc.sync.dma_start(bk_sb, bk[:].rearrange("(t p) -> p t", p=P))

            bv_row = small.tile([1, DLOC], F32)
            nc.sync.dma_start(bv_row, bv[:].rearrange("(a d) -> a d", a=1))
            bv_bc = persist.tile([P, DLOC], F32)
            nc.gpsimd.partition_broadcast(bv_bc, bv_row)

            bo_row = small.tile([1, D], F32)
            nc.sync.dma_start(bo_row, bo[:].rearrange("(a d) -> a d", a=1))
            bo_bc = persist.tile([P, D], F32)
            nc.gpsimd.partition_broadcast(bo_bc, bo_row)

            # staircase causal mask: mask[kk, c] = 1 iff c >= kk + 384;
            # view [:, 384-o : 384-o+width] gives "keep iff qq >= kk + o"
            mask_sb = persist.tile([P, QB + 384], BF16)
            nc.gpsimd.memset(mask_sb, 1.0)
            nc.gpsimd.affine_select(
                out=mask_sb,
                in_=mask_sb,
                compare_op=Alu.is_ge,
                fill=0.0,
                base=-384,
                pattern=[[1, QB + 384]],
                channel_multiplier=-1,
            )

            # ---- QKV projection helpers ----
            from concourse.tile_rust import add_dep_helper as _adh

            qT_sb = persist.tile([P, 2, N], BF16)
            kT_sb = persist.tile([P, 2, N], BF16)
            v_sb = persist.tile([P, NT, 4 * 65], BF16)
            nc.vector.memset(v_sb, 1.0)  # preset ones columns for denominators
            ones_sb = persist.tile([1, 64], F32)
            nc.vector.memset(ones_sb, 1.0)

            def qk_proj(mt):
                for qc in range(NQB):
                    psq = psA.tile([P, QB], F32, tag="proj", name="psq")
                    for kt in range(KT_X):
                        nc.tensor.matmul(
                            psq,
                            lhsT=wq_sb[:, kt, mt * P:(mt + 1) * P],
                            rhs=xT_sb[:, kt, qc * QB:(qc + 1) * QB],
                            start=(kt == 0),
                            stop=(kt == KT_X - 1),
                        )
                    nc.vector.tensor_scalar(
                        qT_sb[:, mt, qc * QB:(qc + 1) * QB],
                        psq,
                        SCALE,
                        bqs_sb[:, mt:mt + 1],
                        Alu.mult,
                        Alu.add,
                    )
                    psk = psA.tile([P, QB], F32, tag="proj", name="psk")
                    for kt in range(KT_X):
                        nc.tensor.matmul(
                            psk,
                            lhsT=wk_sb[:, kt, mt * P:(mt + 1) * P],
                            rhs=xT_sb[:, kt, qc * QB:(qc + 1) * QB],
                            start=(kt == 0),
                            stop=(kt == KT_X - 1),
                        )
                    nc.vector.tensor_scalar_add(
                        kT_sb[:, mt, qc * QB:(qc + 1) * QB],
                        psk,
                        bk_sb[:, mt:mt + 1],
                    )

            def v_proj():
                for nt in range(NT):
                    psv_full = psA.tile([P, QB], F32, tag="proj", name="psv")
                    psv = psv_full[:, :DLOC]
                    for kt in range(KT_X):
                        nc.tensor.matmul(
                            psv,
                            lhsT=xT_sb[:, kt, nt * P:(nt + 1) * P],
                            rhs=wv_sb[:, kt, :],
                            start=(kt == 0),
                            stop=(kt == KT_X - 1),
                        )
                    nc.vector.tensor_tensor(
                        v_sb[:, nt].rearrange("p (h x) -> p h x", x=65)[:, :, 0:64],
                        psv.rearrange("p (h x) -> p h x", x=64),
                        bv_bc.rearrange("p (h x) -> p h x", x=64),
                        Alu.add,
                    )

            # per-core dynamic sequence offset for the output projection
            qoff_sb = small.tile([1, 1], mybir.dt.uint32)
            nc.sync.dma_start(qoff_sb, qoff[:])
            qregs = nc.alloc_registers()
            nc.regs_load(qregs, qoff_sb[0:1, 0:1])
            qoff_sv = nc.snap(qregs, donate=True)

            # one AllGather per head pair so the first overlaps the second
            # pair's attention.  zallX rows: rank r block = global dims
            # [r*256 + pair*128, +128).
            zin = [dram.tile([P, N], BF16, name=f"zin{pr}") for pr in range(2)]
            zall = [dram.tile([TP * P, N], BF16, name=f"zall{pr}")
                    for pr in range(2)]

            # ---- attention for one head pair ----
            def attention(pr, qbs):
                for qb in qbs:
                    kt_max = (qb + 1) * 4
                    zps = [psZ.tile([65, QB], F32, tag="z", name=f"zp{hi}")
                           for hi in range(2)]
                    # software-pipelined emission: the z matmuls of k-tile
                    # kt-1 are emitted after the score pair of k-tile kt, so
                    # the two score matmuls sit adjacent in the PE stream and
                    # run concurrently on disjoint row groups (contraction
                    # partitions 0-63 / 64-127)
                    pending_z = []
                    for kt in range(kt_max):
                        diag = kt >= qb * 4
                        o = kt * P - qb * QB if diag else 0
                        sps, s_insts = [], []
                        for hi in range(2):
                            spf = psS.tile([P, QB], F32, tag="score", name="spf")
                            sp = spf[:, o:QB]
                            si = nc.tensor.matmul(
                                sp,
                                lhsT=kT_sb[hi * 64:(hi + 1) * 64, pr,
                                           kt * P:(kt + 1) * P],
                                rhs=qT_sb[hi * 64:(hi + 1) * 64, pr,
                                          qb * QB + o:(qb + 1) * QB],
                                start=True,
                                stop=True,
                                tile_position=(hi * 64, 0),
                            )
                            sps.append(sp)
                            s_insts.append(si)
                        for args in pending_z:
                            zi = nc.tensor.matmul(**args)
                            # pin the static PE order to [S0,S1,z,z] so the
                            # two score matmuls stay adjacent and overlap on
                            # their disjoint row groups
                            _adh(zi.ins, s_insts[-1].ins, sync=False,
                                 reason="z after score pair")
                        pending_z = []
                        for hi in range(2):
                            wt = wtp.tile([P, QB], BF16, tag="wt", name="wt")
                            nc.scalar.activation(wt[:, o:QB], sps[hi], Act.Exp)
                            if diag:
                                # only the o..o+128 strip straddles the diagonal
                                nc.vector.tensor_tensor(
                                    wt[:, o:o + P],
                                    wt[:, o:o + P],
                                    mask_sb[:, 384:384 + P],
                                    Alu.mult,
                                )
                            pending_z.append(dict(
                                out=zps[hi][:, o:QB],
                                lhsT=v_sb[:, kt,
                                          (2 * pr + hi) * 65:(2 * pr + hi + 1) * 65],
                                rhs=wt[:, o:QB],
                                start=(kt == 0),
                                stop=(kt == kt_max - 1),
                                skip_group_check=True,
                            ))
                    for args in pending_z:
                        nc.tensor.matmul(**args)
                    for hi in range(2):
                        # denominator -> SBUF (approx_fast misbehaves on a
                        # PSUM source), reciprocal, gpsimd partition
                        # broadcast, normalize.  No PE in this chain, and
                        # the zin write rides the gpsimd queue so the sync
                        # queue's post-collective DMAs can't block it.
                        den = small.tile([1, QB], F32, tag="den", name="den")
                        nc.vector.tensor_copy(den, zps[hi][64:65, :])
                        recip = small.tile([1, QB], F32, tag="recip", name="recip")
                        nc.vector.reciprocal_approx_fast(recip, den)
                        rb = small.tile([64, QB], F32, tag="rb", name="rb")
                        nc.gpsimd.partition_broadcast(rb, recip)
                        zn = small.tile([64, QB], BF16, tag="zn", name="zn")
                        nc.vector.tensor_tensor(zn, zps[hi][0:64, :], rb, Alu.mult)
                        nc.gpsimd.dma_start(
                            zin[pr][hi * 64:(hi + 1) * 64,
                                    qb * QB:(qb + 1) * QB],
                            zn,
                        )

            def gather(pr):
                return nc.gpsimd.collective_compute(
                    "AllGather",
                    Alu.bypass,
                    replica_groups=[[0, 1, 2, 3], [4, 5, 6, 7]],
                    ins=[zin[pr].opt()],
                    outs=[zall[pr].opt()],
                )

            # emit in an order that lets the scheduler overlap PE-heavy
            # projection work with the ACT-bound attention phase, and the
            # first AllGather with the second pair's attention
            qk_proj(0)
            v_proj()
            attention(0, [0, 1, 2, 3])
            gather(0)
            qk_proj(1)
            attention(1, [0, 1, 2, 3])
            g1 = gather(1)

            # ---- output projection for this core's 512-row slice ----
            # Split by k-tile parity: even k-tiles only need the pair-0
            # AllGather, so that half runs while the pair-1 collective is
            # still in flight; the odd half + combine follows it.
            wo_sb = persist.tile([P, KT_X, D], BF16)
            nc.sync.dma_start(wo_sb, wo[:].rearrange("(kt p) m -> p kt m", p=P))
            zg_sb = persist.tile([P, KT_X, QB], BF16)
            stage_sb = persist.tile([P, NSLICE // P, D // QB, QB], F32)
            from concourse.bass import ds
            from concourse.tile_rust import add_dep_helper
            # even k-tiles (pair-0 gather) first so the AG#2-gated odd DMAs
            # don't block them on the in-order sync queue
            for kt in [0, 2, 4, 6, 1, 3, 5, 7]:
                zgd = nc.sync.dma_start(
                    zg_sb[:, kt],
                    zall[kt % 2][(kt // 2) * P:(kt // 2 + 1) * P,
                                 ds(qoff_sv, QB)],
                )
                # scheduling-order-only edge: keep these AG-gated DMAs from
                # being placed ahead of attention(1) in the static order,
                # which would serialize attention behind the collective via
                # shared DMA-semaphore counts
                add_dep_helper(zgd.ins, g1.ins, sync=False,
                               reason="zg after gather(1) trigger")
            for mt in range(NSLICE // P):
                for oc in range(D // QB):
                    pse = psA.tile([P, QB], F32, tag="proj", name="pse")
                    for i, kt in enumerate(range(0, KT_X, 2)):
                        nc.tensor.matmul(
                            pse,
                            lhsT=zg_sb[:, kt, mt * P:(mt + 1) * P],
                            rhs=wo_sb[:, kt, oc * QB:(oc + 1) * QB],
                            start=(i == 0),
                            stop=(kt == KT_X - 2),
                        )
                    nc.vector.tensor_copy(stage_sb[:, mt, oc], pse)
            for mt in range(NSLICE // P):
                for oc in range(D // QB):
                    pso = psA.tile([P, QB], F32, tag="proj", name="pso")
                    for i, kt in enumerate(range(1, KT_X, 2)):
                        nc.tensor.matmul(
                            pso,
                            lhsT=zg_sb[:, kt, mt * P:(mt + 1) * P],
                            rhs=wo_sb[:, kt, oc * QB:(oc + 1) * QB],
                            start=(i == 0),
                            stop=(kt == KT_X - 1),
                        )
                    osb = small.tile([P, QB], F32, tag="osb", name="osb")
                    nc.vector.tensor_tensor(
                        osb, pso, stage_sb[:, mt, oc], Alu.add
                    )
                    nc.vector.tensor_tensor(
                        osb, osb, bo_bc[:, oc * QB:(oc + 1) * QB], Alu.add
                    )
                    nc.sync.dma_start(
                        out[mt * P:(mt + 1) * P, oc * QB:(oc + 1) * QB], osb
                    )
    nc.compile()
    return nc


def make_in_maps(inputs):
    x = np.asarray(inputs["inputs"], dtype=np.float32)
    ws = {k: np.asarray(inputs[k], dtype=np.float32) for k in
          ("Wq", "Wk", "Wv", "Wo", "bq", "bk", "bv", "bo")}
    wo_bf = np.ascontiguousarray(ws["Wo"]).astype(BF)
    xT_bf = [np.ascontiguousarray(x[b].T).astype(BF) for b in range(B)]
    in_maps = []
    for c in range(NCORES):
        b, q = c // TP, c % TP
        cols = slice(q * DLOC, (q + 1) * DLOC)
        in_maps.append({
            "xT": xT_bf[b],
            "wq": np.ascontiguousarray(ws["Wq"][:, cols]).astype(BF),
            "wk": np.ascontiguousarray(ws["Wk"][:, cols]).astype(BF),
            "wv": np.ascontiguousarray(ws["Wv"][:, cols]).astype(BF),
            "wo": wo_bf,
            "bq": np.ascontiguousarray(ws["bq"][cols]),
            "bk": np.ascontiguousarray(ws["bk"][cols]),
            "bv": np.ascontiguousarray(ws["bv"][cols]),
            "bo": ws["bo"],
            "qoff": np.array([[q * NSLICE]], dtype=np.uint32),
        })
    return in_maps


def assemble(results):
    outs = [np.asarray(r["out"], dtype=np.float32) for r in results]
    return np.stack(
        [np.concatenate(outs[b * TP:(b + 1) * TP], axis=0) for b in range(B)]
    )


def _ensure_ntff_hook():
    """bass_utils hard-imports antenv.axon_hooks for trace=True; this image
    lacks it.  Shim it and register the ctypes NTFF hook from trn_boot."""
    import types

    if "antenv.axon_hooks" in sys.modules:
        return
    try:
        import antenv.axon_hooks  # noqa: F401
        return
    except ImportError:
        pass
    mod = types.ModuleType("antenv.axon_hooks")
    mod._hook = None
    mod.set_axon_ntff_profile_hook = lambda h: setattr(mod, "_hook", h)
    mod.get_axon_ntff_profile_hook = lambda: mod._hook
    sys.modules["antenv.axon_hooks"] = mod
    try:
        import antenv
        antenv.axon_hooks = mod
    except Exception:
        pass
    try:
        from trn_agent_boot.trn_boot import _ntff_profile_via_ctypes
        hook = _ntff_profile_via_ctypes("/opt/axon/libaxon_pjrt.so")
        if hook is not None:
            mod._hook = hook
    except Exception:
        pass


_cached_nc = None


def kernel(**inputs):
    global _cached_nc
    _ensure_ntff_hook()
    from concourse.bass_utils import run_bass_kernel_spmd

    if _cached_nc is None:
        _cached_nc = build_bass()
    trace = bool(int(os.environ.get("MHA_TRACE", "0")))
    res = run_bass_kernel_spmd(
        _cached_nc, make_in_maps(inputs), core_ids=list(range(NCORES)),
        trace=trace,
    )
    if trace and res.exec_time_ns is not None:
        print(f"HW exec time: {res.exec_time_ns} ns")
        kernel.last_exec_time_ns = res.exec_time_ns
    return assemble(res.results)
